# revision 5
# baseline (speedup 1.0000x reference)
"""AttnDecoderRNN step on 8 TRN2 NeuronCores (Bass/Tile).

Sharding (per sharding hint): vocab-parallel out projection (and embedding
row handled as a gather), hidden-sharded comb/GRU matmuls, replicated
attention. Collectives: AllGather of x (post-comb relu), AllGather of h',
AllGather of per-core log-softmax stats (max, sumexp).

Shapes: NHID=1024, NOUT=50257, MAX_LEN=24, batch=1.
Per-core vocab shard: VS=6400 (8*6400=51200 >= 50257; padding gets bias -1e4).
"""
import os
import sys
import types
import contextlib
import ctypes

import numpy as np

# ---------------------------------------------------------------------------
# antenv.axon_hooks shim: the container's antenv stub lacks this module, but
# concourse.bass_utils imports it when tracing is requested (BASS_TRACE=1).
# Provide it, with the ctypes NTFF profile hook libaxon exposes.
# ---------------------------------------------------------------------------
_HOOK = [None]


def _install_axon_hook_shim():
    if "antenv.axon_hooks" not in sys.modules:
        mod = types.ModuleType("antenv.axon_hooks")

        def set_axon_ntff_profile_hook(h):
            _HOOK[0] = h

        def get_axon_ntff_profile_hook():
            return _HOOK[0]

        mod.set_axon_ntff_profile_hook = set_axon_ntff_profile_hook
        mod.get_axon_ntff_profile_hook = get_axon_ntff_profile_hook
        sys.modules["antenv.axon_hooks"] = mod
        try:
            import antenv

            antenv.axon_hooks = mod
        except ImportError:
            pass
    if _HOOK[0] is None:
        so_path = "/opt/axon/libaxon_pjrt.so"
        try:
            lib = ctypes.CDLL(so_path)
        except OSError:
            return
        if not hasattr(lib, "axon_start_nrt_profile"):
            return
        lib.axon_start_nrt_profile.argtypes = [
            ctypes.POINTER(ctypes.c_int64),
            ctypes.c_size_t,
        ]
        lib.axon_start_nrt_profile.restype = ctypes.c_int64
        lib.axon_stop_nrt_profile.argtypes = [ctypes.c_char_p]
        lib.axon_stop_nrt_profile.restype = ctypes.c_int64

        @contextlib.contextmanager
        def _hook(output_dir, device_ids):
            import jax

            jax.devices()
            if device_ids:
                ids = (ctypes.c_int64 * len(device_ids))(*device_ids)
                rc = lib.axon_start_nrt_profile(ids, len(device_ids))
            else:
                rc = lib.axon_start_nrt_profile(None, 0)
            if rc != 0:
                raise RuntimeError(f"axon_start_nrt_profile rc={rc}")
            try:
                yield
            finally:
                n = lib.axon_stop_nrt_profile(str(output_dir).encode())
                print(f"profile: {n} file(s) -> {output_dir}", file=sys.stderr)

        sys.modules["antenv.axon_hooks"].set_axon_ntff_profile_hook(_hook)


_install_axon_hook_shim()

NCORES = 8
NHID = 1024
NOUT = 50257
MAX_LEN = 24
HC = NHID // 128          # 8 hidden chunks of 128
VS = 6400                 # vocab rows per core (padded)
TN = 400                  # out-projection free-dim tile
NT = VS // TN             # 16 tiles per core
PAD_BIAS = -1.0e4         # bias on padded vocab rows: exp() underflows to 0

_CACHE = {}


def _build():
    import concourse.bass as bass
    import concourse.tile as tile
    from concourse import bacc, mybir, masks
    from contextlib import ExitStack

    f32 = mybir.dt.float32

    nc = bacc.Bacc(
        "TRN2",
        target_bir_lowering=False,
        debug=False,
        enable_asserts=True,
        num_devices=NCORES,
    )

    # ---- I/O ----
    emb_in = nc.dram_tensor("emb_in", [128, HC], f32, kind="ExternalInput")
    h0c_in = nc.dram_tensor("h0c_in", [128, HC], f32, kind="ExternalInput")
    h0own_in = nc.dram_tensor("h0own_in", [128, 1], f32, kind="ExternalInput")
    enc_in = nc.dram_tensor("enc_in", [MAX_LEN, NHID], f32, kind="ExternalInput")
    attnw_in = nc.dram_tensor("attnw_in", [128, 16 * MAX_LEN], f32, kind="ExternalInput")
    attnb_in = nc.dram_tensor("attnb_in", [1, MAX_LEN], f32, kind="ExternalInput")
    combw_in = nc.dram_tensor("combw_in", [128, 16 * 128], f32, kind="ExternalInput")
    combb_in = nc.dram_tensor("combb_in", [128, 1], f32, kind="ExternalInput")
    wih_in = nc.dram_tensor("wih_in", [128, 3 * HC * 128], f32, kind="ExternalInput")
    whh_in = nc.dram_tensor("whh_in", [128, 3 * HC * 128], f32, kind="ExternalInput")
    bih_in = nc.dram_tensor("bih_in", [128, 3], f32, kind="ExternalInput")
    bhh_in = nc.dram_tensor("bhh_in", [128, 3], f32, kind="ExternalInput")
    wout_in = nc.dram_tensor("wout_in", [NT, 128, HC * TN], f32, kind="ExternalInput")
    bout_in = nc.dram_tensor("bout_in", [16, TN], f32, kind="ExternalInput")

    logp_out = nc.dram_tensor("logp_out", [16, TN], f32, kind="ExternalOutput")
    h_out = nc.dram_tensor("h_out", [128, 1], f32, kind="ExternalOutput")
    attn_out = nc.dram_tensor("attn_out", [1, MAX_LEN], f32, kind="ExternalOutput")

    RG = [list(range(NCORES))]

    with tile.TileContext(nc) as tc:
        with ExitStack() as ctx:
            wpool = ctx.enter_context(tc.tile_pool(name="wpool", bufs=8))
            cpool = ctx.enter_context(tc.tile_pool(name="cpool", bufs=1))
            spool = ctx.enter_context(tc.tile_pool(name="spool", bufs=2))
            pp = ctx.enter_context(tc.tile_pool(name="pp", bufs=2, space="PSUM"))
            dram = ctx.enter_context(tc.tile_pool(name="dram", bufs=1, space="DRAM"))

            # ---- constants / small inputs to SBUF ----
            ident = cpool.tile([128, 128], f32)
            masks.make_identity(nc, ident[:])
            ones_row = cpool.tile([1, 128], f32)   # [1,P] for broadcasts (lhsT)
            nc.gpsimd.memset(ones_row[:], 1.0)
            ones_col = cpool.tile([128, 1], f32)   # [P,1] for partition sums (rhs)
            nc.gpsimd.memset(ones_col[:], 1.0)

            emb_sb = cpool.tile([128, HC], f32)
            nc.gpsimd.dma_start(emb_sb[:], emb_in[:])
            h0c_sb = cpool.tile([128, HC], f32)
            nc.gpsimd.dma_start(h0c_sb[:], h0c_in[:])
            h0own_sb = cpool.tile([128, 1], f32)
            nc.gpsimd.dma_start(h0own_sb[:], h0own_in[:])
            enc_sb = cpool.tile([MAX_LEN, NHID], f32)
            nc.gpsimd.dma_start(enc_sb[:], enc_in[:])
            attnw_sb = cpool.tile([128, 16 * MAX_LEN], f32)
            nc.gpsimd.dma_start(attnw_sb[:], attnw_in[:])
            attnb_sb = cpool.tile([1, MAX_LEN], f32)
            nc.gpsimd.dma_start(attnb_sb[:], attnb_in[:])
            combw_sb = cpool.tile([128, 16 * 128], f32)
            nc.gpsimd.dma_start(combw_sb[:], combw_in[:])
            combb_sb = cpool.tile([128, 1], f32)
            nc.gpsimd.dma_start(combb_sb[:], combb_in[:])
            wih_sb = cpool.tile([128, 3 * HC * 128], f32)
            nc.gpsimd.dma_start(wih_sb[:], wih_in[:])
            whh_sb = cpool.tile([128, 3 * HC * 128], f32)
            nc.gpsimd.dma_start(whh_sb[:], whh_in[:])
            bih_sb = cpool.tile([128, 3], f32)
            nc.gpsimd.dma_start(bih_sb[:], bih_in[:])
            bhh_sb = cpool.tile([128, 3], f32)
            nc.gpsimd.dma_start(bhh_sb[:], bhh_in[:])
            bout_sb = cpool.tile([16, TN], f32)
            nc.gpsimd.dma_start(bout_sb[:], bout_in[:])

            # ---- attention (replicated) ----
            # attn_logits[1,24] = cat(emb, h0) @ attn_W.T
            psA = pp.tile([128, MAX_LEN], f32, tag="psA", bufs=1)
            alog_ps = psA[0:1, 0:MAX_LEN]
            for c in range(HC):
                nc.tensor.matmul(
                    alog_ps,
                    emb_sb[:, c : c + 1],
                    attnw_sb[:, c * MAX_LEN : (c + 1) * MAX_LEN],
                    start=(c == 0),
                    stop=False,
                )
            for c in range(HC):
                nc.tensor.matmul(
                    alog_ps,
                    h0c_sb[:, c : c + 1],
                    attnw_sb[:, (HC + c) * MAX_LEN : (HC + c + 1) * MAX_LEN],
                    start=False,
                    stop=(c == HC - 1),
                )
            alog_sb = spool.tile([1, MAX_LEN], f32)
            nc.vector.tensor_add(alog_sb[:], alog_ps, attnb_sb[:])
            amax = spool.tile([1, 1], f32)
            nc.vector.reduce_max(amax[:], alog_sb[:], axis=mybir.AxisListType.X)
            namax = spool.tile([1, 1], f32)
            nc.scalar.mul(namax[:], amax[:], -1.0)
            probs = spool.tile([1, MAX_LEN], f32)
            sume = spool.tile([1, 1], f32)
            nc.scalar.activation(
                probs[:], alog_sb[:], mybir.ActivationFunctionType.Exp,
                bias=namax[0:1, 0:1], accum_out=sume[:],
            )
            rinv = spool.tile([1, 1], f32)
            nc.vector.reciprocal(rinv[:], sume[:])
            attnp_sb = spool.tile([1, MAX_LEN], f32)
            nc.scalar.mul(attnp_sb[:], probs[:], rinv[0:1, 0:1])
            nc.gpsimd.dma_start(attn_out[:], attnp_sb[:])

            # transpose attn probs -> [24, 1]
            psS = pp.tile([128, 16], f32, tag="psS", bufs=2)
            nc.tensor.transpose(psS[0:MAX_LEN, 0:1], attnp_sb[:], ident[0:1, 0:1])
            awt_sb = spool.tile([MAX_LEN, 1], f32)
            nc.vector.tensor_copy(awt_sb[:], psS[0:MAX_LEN, 0:1])

            # attn_applied chunks: [128, HC]
            psA2 = pp.tile([128, MAX_LEN], f32, tag="psA", bufs=1)
            for c in range(HC):
                nc.tensor.matmul(
                    psA2[:, c : c + 1],
                    enc_sb[0:MAX_LEN, c * 128 : (c + 1) * 128],
                    awt_sb[:],
                    start=True,
                    stop=True,
                )
            aap_sb = spool.tile([128, HC], f32)
            nc.vector.tensor_copy(aap_sb[:], psA2[:, 0:HC])

            # ---- comb (sharded): x_shard = relu(cat(emb, aap) @ comb_W_sh.T + b) ----
            psC = pp.tile([128, 1], f32, tag="psG", bufs=2)
            for c in range(HC):
                nc.tensor.matmul(
                    psC[:],
                    combw_sb[:, c * 128 : (c + 1) * 128],
                    emb_sb[:, c : c + 1],
                    start=(c == 0),
                    stop=False,
                )
            for c in range(HC):
                nc.tensor.matmul(
                    psC[:],
                    combw_sb[:, (HC + c) * 128 : (HC + c + 1) * 128],
                    aap_sb[:, c : c + 1],
                    start=False,
                    stop=(c == HC - 1),
                )
            xsh_sb = spool.tile([128, 1], f32)
            nc.scalar.activation(
                xsh_sb[:], psC[:], mybir.ActivationFunctionType.Relu,
                bias=combb_sb[:, 0:1],
            )

            # ---- AllGather x ----
            xa_in = dram.tile([128, 1], f32)
            xa_out = dram.tile([NHID, 1], f32, addr_space="Shared")
            nc.gpsimd.dma_start(xa_in[:], xsh_sb[:])
            nc.gpsimd.collective_compute(
                "AllGather", mybir.AluOpType.bypass, replica_groups=RG,
                ins=[xa_in[:].opt()], outs=[xa_out[:].opt()],
            )
            x_sb = cpool.tile([128, HC], f32)
            nc.gpsimd.dma_start(
                x_sb[:], xa_out[:].rearrange("(c p) o -> p (c o)", p=128)
            )

            # ---- GRU (sharded) ----
            def gate_mms(ps, g, rhs_x, rhs_h):
                n_mm = (HC if rhs_x is not None else 0) + (HC if rhs_h is not None else 0)
                i = 0
                if rhs_x is not None:
                    for k in range(HC):
                        nc.tensor.matmul(
                            ps,
                            wih_sb[:, (g * HC + k) * 128 : (g * HC + k + 1) * 128],
                            rhs_x[:, k : k + 1],
                            start=(i == 0), stop=(i == n_mm - 1),
                        )
                        i += 1
                if rhs_h is not None:
                    for k in range(HC):
                        nc.tensor.matmul(
                            ps,
                            whh_sb[:, (g * HC + k) * 128 : (g * HC + k + 1) * 128],
                            rhs_h[:, k : k + 1],
                            start=(i == 0), stop=(i == n_mm - 1),
                        )
                        i += 1

            # r and z: gi + gh fused in one accumulation group
            psR = pp.tile([128, 1], f32, tag="psG", bufs=2)
            i = 0
            for k in range(HC):
                nc.tensor.matmul(
                    psR[:], wih_sb[:, (0 * HC + k) * 128 : (0 * HC + k + 1) * 128],
                    x_sb[:, k : k + 1], start=(i == 0), stop=False)
                i += 1
            for k in range(HC):
                nc.tensor.matmul(
                    psR[:], whh_sb[:, (0 * HC + k) * 128 : (0 * HC + k + 1) * 128],
                    h0c_sb[:, k : k + 1], start=False, stop=(k == HC - 1))
            brz_sb = spool.tile([128, 2], f32)
            nc.vector.tensor_add(brz_sb[:], bih_sb[:, 0:2], bhh_sb[:, 0:2])
            r_sb = spool.tile([128, 1], f32)
            nc.scalar.activation(
                r_sb[:], psR[:], mybir.ActivationFunctionType.Sigmoid,
                bias=brz_sb[:, 0:1])

            psZ = pp.tile([128, 1], f32, tag="psG", bufs=2)
            i = 0
            for k in range(HC):
                nc.tensor.matmul(
                    psZ[:], wih_sb[:, (1 * HC + k) * 128 : (1 * HC + k + 1) * 128],
                    x_sb[:, k : k + 1], start=(i == 0), stop=False)
                i += 1
            for k in range(HC):
                nc.tensor.matmul(
                    psZ[:], whh_sb[:, (1 * HC + k) * 128 : (1 * HC + k + 1) * 128],
                    h0c_sb[:, k : k + 1], start=False, stop=(k == HC - 1))
            z_sb = spool.tile([128, 1], f32)
            nc.scalar.activation(
                z_sb[:], psZ[:], mybir.ActivationFunctionType.Sigmoid,
                bias=brz_sb[:, 1:2])

            psIN = pp.tile([128, 1], f32, tag="psG", bufs=2)
            for k in range(HC):
                nc.tensor.matmul(
                    psIN[:], wih_sb[:, (2 * HC + k) * 128 : (2 * HC + k + 1) * 128],
                    x_sb[:, k : k + 1], start=(k == 0), stop=(k == HC - 1))
            psHN = pp.tile([128, 1], f32, tag="psG", bufs=2)
            for k in range(HC):
                nc.tensor.matmul(
                    psHN[:], whh_sb[:, (2 * HC + k) * 128 : (2 * HC + k + 1) * 128],
                    h0c_sb[:, k : k + 1], start=(k == 0), stop=(k == HC - 1))

            hnb_sb = spool.tile([128, 1], f32)
            nc.scalar.activation(
                hnb_sb[:], psHN[:], mybir.ActivationFunctionType.Identity,
                bias=bhh_sb[:, 2:3])
            rhn_sb = spool.tile([128, 1], f32)
            nc.vector.tensor_mul(rhn_sb[:], r_sb[:], hnb_sb[:])
            t1_sb = spool.tile([128, 1], f32)
            nc.vector.tensor_add(t1_sb[:], psIN[:], rhn_sb[:])
            n_sb = spool.tile([128, 1], f32)
            nc.scalar.activation(
                n_sb[:], t1_sb[:], mybir.ActivationFunctionType.Tanh,
                bias=bih_sb[:, 2:3])
            d_sb = spool.tile([128, 1], f32)
            nc.vector.tensor_sub(d_sb[:], h0own_sb[:], n_sb[:])
            zd_sb = spool.tile([128, 1], f32)
            nc.vector.tensor_mul(zd_sb[:], z_sb[:], d_sb[:])
            hn_sb = spool.tile([128, 1], f32)
            nc.vector.tensor_add(hn_sb[:], n_sb[:], zd_sb[:])
            nc.gpsimd.dma_start(h_out[:], hn_sb[:])

            # ---- AllGather h' ----
            ha_in = dram.tile([128, 1], f32)
            ha_out = dram.tile([NHID, 1], f32, addr_space="Shared")
            nc.gpsimd.dma_start(ha_in[:], hn_sb[:])
            nc.gpsimd.collective_compute(
                "AllGather", mybir.AluOpType.bypass, replica_groups=RG,
                ins=[ha_in[:].opt()], outs=[ha_out[:].opt()],
            )
            h_sb = cpool.tile([128, HC], f32)
            nc.gpsimd.dma_start(
                h_sb[:], ha_out[:].rearrange("(c p) o -> p (c o)", p=128)
            )

            # ---- out projection (streamed, vocab shard VS=6400, 16 tiles) ----
            logits0_sb = cpool.tile([16, TN], f32)
            for t in range(NT):
                w_tile = wpool.tile([128, HC * TN], f32, tag="wtile")
                nc.sync.dma_start(w_tile[:], wout_in[t])
                psT = pp.tile([1, TN], f32, tag="psT", bufs=2)
                for k in range(HC):
                    nc.tensor.matmul(
                        psT[:],
                        h_sb[:, k : k + 1],
                        w_tile[:, k * TN : (k + 1) * TN],
                        start=(k == 0),
                        stop=(k == HC - 1),
                    )
                # compute engines can't address partition t directly (32-part
                # alignment rule) — stage on partition 0, DMA-scatter to row t
                lrow = spool.tile([1, TN], f32, tag="lrow", bufs=3)
                nc.scalar.copy(lrow[:], psT[:])
                nc.gpsimd.dma_start(logits0_sb[t : t + 1, :], lrow[:])
            # bias add (also applies the -1e4 padding bias)
            logits_sb = cpool.tile([16, TN], f32)
            nc.vector.tensor_add(logits_sb[:], logits0_sb[:], bout_sb[:])

            # ---- local log-softmax stats ----
            mx16 = spool.tile([16, 1], f32)
            nc.vector.reduce_max(mx16[:], logits_sb[:], axis=mybir.AxisListType.X)
            psS2 = pp.tile([128, 16], f32, tag="psS", bufs=2)
            nc.tensor.transpose(psS2[0:1, 0:16], mx16[:], ident[0:16, 0:16])
            mt_sb = spool.tile([1, 16], f32)
            nc.vector.tensor_copy(mt_sb[:], psS2[0:1, 0:16])
            mc = spool.tile([1, 1], f32)
            nc.vector.reduce_max(mc[:], mt_sb[:], axis=mybir.AxisListType.X)
            nmc = spool.tile([1, 1], f32)
            nc.scalar.mul(nmc[:], mc[:], -1.0)
            psB = pp.tile([16, 1], f32, tag="psS", bufs=2)
            nc.tensor.matmul(psB[:], ones_row[0:1, 0:16], nmc[:], start=True, stop=True)
            nm16_sb = spool.tile([16, 1], f32)
            nc.vector.tensor_copy(nm16_sb[:], psB[:])
            e16 = spool.tile([16, TN], f32)
            zrow = spool.tile([16, 1], f32)
            nc.scalar.activation(
                e16[:], logits_sb[:], mybir.ActivationFunctionType.Exp,
                bias=nm16_sb[:, 0:1], accum_out=zrow[:],
            )
            psZc = pp.tile([1, 1], f32, tag="psS", bufs=2)
            nc.tensor.matmul(psZc[:], zrow[:], ones_col[0:16, 0:1], start=True, stop=True)
            stats_sb = spool.tile([1, 2], f32)
            nc.vector.tensor_copy(stats_sb[0:1, 0:1], mc[:])
            nc.vector.tensor_copy(stats_sb[0:1, 1:2], psZc[:])

            # ---- AllGather stats ----
            st_in = dram.tile([1, 2], f32)
            st_out = dram.tile([NCORES, 2], f32, addr_space="Shared")
            nc.gpsimd.dma_start(st_in[:], stats_sb[:])
            nc.gpsimd.collective_compute(
                "AllGather", mybir.AluOpType.bypass, replica_groups=RG,
                ins=[st_in[:].opt()], outs=[st_out[:].opt()],
            )
            s8_sb = spool.tile([NCORES, 2], f32)
            nc.gpsimd.dma_start(s8_sb[:], st_out[:])

            # global max M
            psM = pp.tile([128, 16], f32, tag="psS", bufs=2)
            nc.tensor.transpose(psM[0:1, 0:NCORES], s8_sb[:, 0:1], ident[0:NCORES, 0:NCORES])
            m1_sb = spool.tile([1, NCORES], f32)
            nc.vector.tensor_copy(m1_sb[:], psM[0:1, 0:NCORES])
            gM = spool.tile([1, 1], f32)
            nc.vector.reduce_max(gM[:], m1_sb[:], axis=mybir.AxisListType.X)
            ngM = spool.tile([1, 1], f32)
            nc.scalar.mul(ngM[:], gM[:], -1.0)
            psB2 = pp.tile([NCORES, 1], f32, tag="psS", bufs=2)
            nc.tensor.matmul(psB2[:], ones_row[0:1, 0:NCORES], ngM[:], start=True, stop=True)
            ngM8_sb = spool.tile([NCORES, 1], f32)
            nc.vector.tensor_copy(ngM8_sb[:], psB2[:])
            # Z = sum_c Z_c * exp(m_c - M)
            e8 = spool.tile([NCORES, 1], f32)
            nc.scalar.activation(
                e8[:], s8_sb[:, 0:1], mybir.ActivationFunctionType.Exp,
                bias=ngM8_sb[:, 0:1])
            s8p = spool.tile([NCORES, 1], f32)
            nc.vector.tensor_mul(s8p[:], e8[:], s8_sb[:, 1:2])
            psZg = pp.tile([1, 1], f32, tag="psS", bufs=2)
            nc.tensor.matmul(psZg[:], s8p[:], ones_col[0:NCORES, 0:1], start=True, stop=True)
            lnZ = spool.tile([1, 1], f32)
            nc.scalar.activation(lnZ[:], psZg[:], mybir.ActivationFunctionType.Ln)
            C = spool.tile([1, 1], f32)
            nc.scalar.activation(
                C[:], lnZ[:], mybir.ActivationFunctionType.Identity,
                bias=gM[0:1, 0:1])
            nC = spool.tile([1, 1], f32)
            nc.scalar.mul(nC[:], C[:], -1.0)
            psB3 = pp.tile([16, 1], f32, tag="psS", bufs=2)
            nc.tensor.matmul(psB3[:], ones_row[0:1, 0:16], nC[:], start=True, stop=True)
            nC16_sb = spool.tile([16, 1], f32)
            nc.vector.tensor_copy(nC16_sb[:], psB3[:])
            logp_sb = spool.tile([16, TN], f32)
            nc.scalar.activation(
                logp_sb[:], logits_sb[:], mybir.ActivationFunctionType.Identity,
                bias=nC16_sb[:, 0:1])
            nc.sync.dma_start(logp_out[:], logp_sb[:])

    nc.compile()
    return nc


def _prep_inputs(inp, hidden, encoder_outputs, emb_W, attn_W, attn_b,
                 comb_W, comb_b, W_ih, W_hh, b_ih, b_hh, out_W, out_b):
    """Shard/layout the full inputs into 8 per-core input maps."""
    f = np.float32
    idx = int(np.asarray(inp).ravel()[0])
    emb_row = np.asarray(emb_W[idx], dtype=f)                 # [1024]
    h0 = np.asarray(hidden, dtype=f).ravel()                  # [1024]
    enc = np.ascontiguousarray(np.asarray(encoder_outputs, dtype=f))  # [24,1024]

    def chunked_vec(v):
        # [1024] -> [128, 8] with [p, c] = v[c*128+p]
        return np.ascontiguousarray(v.reshape(HC, 128).T)

    emb_c = chunked_vec(emb_row)
    h0_c = chunked_vec(h0)

    # attn_W [24, 2048] -> [128, 16*24]
    aT = np.asarray(attn_W, dtype=f).T.reshape(16, 128, MAX_LEN)
    attnw = np.ascontiguousarray(aT.transpose(1, 0, 2).reshape(128, 16 * MAX_LEN))
    attnb = np.ascontiguousarray(np.asarray(attn_b, dtype=f).reshape(1, MAX_LEN))

    comb_W = np.asarray(comb_W, dtype=f)
    comb_b_a = np.asarray(comb_b, dtype=f)
    W_ih_a = np.asarray(W_ih, dtype=f)
    W_hh_a = np.asarray(W_hh, dtype=f)
    b_ih_a = np.asarray(b_ih, dtype=f)
    b_hh_a = np.asarray(b_hh, dtype=f)
    out_W_a = np.asarray(out_W, dtype=f)
    out_b_a = np.asarray(out_b, dtype=f)

    in_maps = []
    for j in range(NCORES):
        sl = slice(j * 128, (j + 1) * 128)
        # comb shard [128, 2048] -> [128(p), 16*128]
        cw = comb_W[sl]                                    # [128, 2048]
        cwT = cw.T.reshape(16, 128, 128)                   # [c, p, m]
        combw = np.ascontiguousarray(cwT.transpose(1, 0, 2).reshape(128, 16 * 128))
        combb = np.ascontiguousarray(comb_b_a[sl].reshape(128, 1))

        def gate_pack(W):
            cols = []
            for g in range(3):
                Wg = W[g * NHID + j * 128 : g * NHID + (j + 1) * 128]  # [128, 1024]
                T = Wg.T.reshape(HC, 128, 128)                          # [k, p, m]
                cols.append(T.transpose(1, 0, 2).reshape(128, HC * 128))
            return np.ascontiguousarray(np.concatenate(cols, axis=1))

        wih = gate_pack(W_ih_a)
        whh = gate_pack(W_hh_a)
        bih = np.ascontiguousarray(
            np.stack([b_ih_a[g * NHID + j * 128 : g * NHID + (j + 1) * 128]
                      for g in range(3)], axis=1))
        bhh = np.ascontiguousarray(
            np.stack([b_hh_a[g * NHID + j * 128 : g * NHID + (j + 1) * 128]
                      for g in range(3)], axis=1))

        # out_W vocab shard [VS, 1024] (zero-padded), bias shard with PAD_BIAS
        lo, hi = j * VS, min((j + 1) * VS, NOUT)
        nreal = max(0, hi - lo)
        wsh = np.zeros((VS, NHID), dtype=f)
        bsh = np.full((VS,), PAD_BIAS, dtype=f)
        if nreal > 0:
            wsh[:nreal] = out_W_a[lo:hi]
            bsh[:nreal] = out_b_a[lo:hi]
        WT = wsh.T                                         # [1024, 6400]
        # [NT, 128, HC*TN]: [t, p, k*TN+n] = WT[k*128+p, t*TN+n]
        warr = np.ascontiguousarray(
            WT.reshape(HC, 128, NT, TN).transpose(2, 1, 0, 3).reshape(NT, 128, HC * TN))
        barr = np.ascontiguousarray(bsh.reshape(16, TN))

        in_maps.append({
            "emb_in": emb_c, "h0c_in": h0_c,
            "h0own_in": np.ascontiguousarray(h0[sl].reshape(128, 1)),
            "enc_in": enc, "attnw_in": attnw, "attnb_in": attnb,
            "combw_in": combw, "combb_in": combb,
            "wih_in": wih, "whh_in": whh, "bih_in": bih, "bhh_in": bhh,
            "wout_in": warr, "bout_in": barr,
        })
    return in_maps


def run(trace=False, **inputs):
    from concourse.bass_utils import run_bass_kernel_spmd

    if "nc" not in _CACHE:
        _CACHE["nc"] = _build()
    nc = _CACHE["nc"]

    inputs.pop("encoder_output", None)  # unused by the reference computation
    in_maps = _prep_inputs(**inputs)
    res = run_bass_kernel_spmd(
        nc, in_maps, core_ids=list(range(NCORES)), trace=trace
    )

    logp = np.concatenate(
        [res.results[j]["logp_out"].reshape(-1) for j in range(NCORES)]
    )[:NOUT].reshape(1, NOUT).astype(np.float32)
    h = np.concatenate(
        [res.results[j]["h_out"].reshape(-1) for j in range(NCORES)]
    ).reshape(1, 1, NHID).astype(np.float32)
    attn = res.results[0]["attn_out"].reshape(1, MAX_LEN).astype(np.float32)
    return (logp, h, attn), res


def kernel(**inputs):
    out, _ = run(trace=bool(os.environ.get("KERNEL_TRACE")), **inputs)
    return out
